# revision 6
# baseline (speedup 1.0000x reference)
"""Multi-head causal attention (B=2, S=2048, d_model=1024, H=16) on 8 Trainium2
NeuronCores — chunk-major pipelined version.

Sharding: core c -> batch b = c // 4, head group g = c % 4 (heads 4g..4g+3).
Data-parallel over the batch, tensor-parallel over heads: each core computes
QKV projections for its 4 heads (column-sliced Wqkv), causal attention for
those heads, and a partial output projection (row-sliced Wo). The host sums
the 4 partial outputs per batch and adds the output bias.

Device dataflow (per core): the sequence is processed in 4 query chunks of
512 so input DMA (x column-slices), QKV projection, attention, output
projection, and output stores all overlap chunk-to-chunk:

  prologue: chunk-0 QKV runs k-major, accumulating as x/W k-tiles stream in.
  chunk c:  [attention c] then [QKV c+1 (ch-major) | proj c | stores c].

Attention per (head, chunk): scoresT[j, i] = kT.T @ qT per j-tile pair into
one 2-bank PSUM tile (fp32r), one wide Exp (scale folded) straight to bf16
pT, DVE triangular mask on diagonal blocks, AV (all-bf16, full PE rate at
any width) accumulated over j-tiles into av[65, 512] whose row 64 is the
softmax denominator l via a ones column in v. 1/l comes from a single-
partition DVE reciprocal (no DRAM bounce), broadcast across 64 partitions
with a K=1 matmul; values are normalized straight out of PSUM into bf16.
Odd heads land in partitions 64:128 via an identity-matmul partition move
(no SBUF-to-SBUF DMA). Output projection contracts K=128 per head pair in
bf16; partial outputs stream out per 128-row tile.
"""

import sys

sys.path.insert(0, "/opt/trn_rl_repo")

import numpy as np
from contextlib import nullcontext as _nullcontext

import concourse.bass as bass
import concourse.mybir as mybir
import concourse.tile as tile
from concourse.bass_utils import run_bass_kernel_spmd

F32 = mybir.dt.float32
F32R = mybir.dt.float32r
BF16 = mybir.dt.bfloat16

B, S, D = 2, 2048, 1024
H_TOT = 16
HD = 64
H_PER_CORE = 4
N_CORES = 8
SCALE = 1.0 / np.sqrt(HD)

ST = S // 128   # 16 sequence tiles of 128
NCH = S // 512  # 4 query chunks of 512

# Q/K activations in bf16: logit error ~1e-2 relative on exp weights, well
# inside the 2e-2 budget; buys full-rate narrow matmuls on diagonal tiles,
# halves qkT SBUF and doubles drain throughput.
QK_DT = BF16


def _split_multi_waits(nc):
    """This container's walrus rejects >1 sem wait per instruction. Move
    extra waits onto fresh single-wait NOPs on the same engine, inserted
    immediately before the instruction (same-engine streams are in-order,
    so semantics are unchanged)."""
    n = 0
    for func in nc.m.functions:
        for bb in func.blocks:
            i = 0
            while i < len(bb.instructions):
                ins = bb.instructions[i]
                si = ins.sync_info
                if si is not None and si.on_wait and len(si.on_wait) > 1:
                    waits = list(si.on_wait)
                    si.on_wait = [waits[-1]]
                    eng = nc.engines[ins.engine]
                    nops = []
                    for w in waits[:-1]:
                        ni = eng.nop(nofuse=True, hint="wait_split").ins
                        if ni.sync_info is None:
                            ni.sync_info = mybir.SyncInfo(on_wait=[w], on_update=[])
                        else:
                            ni.sync_info.on_wait = [w]
                        nops.append(ni)
                    for ni in nops:
                        for f2 in nc.m.functions:
                            for bb2 in f2.blocks:
                                if ni in bb2.instructions:
                                    bb2.instructions.remove(ni)
                    for k, ni in enumerate(nops):
                        bb.instructions.insert(i + k, ni)
                    i += len(nops)
                    n += len(nops)
                i += 1
    return n


def _dram_row_bcast(handle, offset_elems, width, parts):
    """AP that broadcasts a DRAM row of `width` elems across `parts` partitions."""
    return bass.AP(tensor=handle, offset=offset_elems, ap=[[0, parts], [1, width]])


def build_bass():
    nc = bass.Bass()

    xT = nc.dram_tensor("xT", [D, S], BF16, kind="ExternalInput")
    w = nc.dram_tensor("w", [D, 768], BF16, kind="ExternalInput")
    bias_qk = nc.dram_tensor("bias_qk", [128, 4], F32, kind="ExternalInput")
    bias_v = nc.dram_tensor("bias_v", [256], F32, kind="ExternalInput")
    wo = nc.dram_tensor("wo", [256, D], BF16, kind="ExternalInput")
    tri = nc.dram_tensor("tri", [128, 128], BF16, kind="ExternalInput")
    eye = nc.dram_tensor("eye", [64, 64], BF16, kind="ExternalInput")
    out = nc.dram_tensor("out", [S, D], BF16, kind="ExternalOutput")

    # Per-queue DMA bandwidth is low in this environment; round-robin bulk
    # transfers across all three DMA-capable queues (SP-HWDGE, ACT-HWDGE,
    # Pool-SWDGE). ACT's queue only carries loads issued while ScalarE is
    # idle plus stores whose staging copy ran on ScalarE (in-order, no
    # waits), so it never stalls the exp stream.
    _dma_engines = [nc.sync, nc.scalar, nc.gpsimd]
    _dma_i = [0]

    def dma_rr(out_ap, in_ap):
        e = _dma_engines[_dma_i[0] % len(_dma_engines)]
        _dma_i[0] += 1
        return e.dma_start(out_ap, in_ap)

    _dma2_engines = [nc.sync, nc.gpsimd]
    _dma2_i = [0]

    def dma_rr2(out_ap, in_ap):
        e = _dma2_engines[_dma2_i[0] % len(_dma2_engines)]
        _dma2_i[0] += 1
        return e.dma_start(out_ap, in_ap)

    with tile.TileContext(nc) as tc:
        with (
            tc.tile_pool(name="consts", bufs=1) as consts,
            tc.tile_pool(name="xw_p", bufs=1) as xw_p,
            tc.tile_pool(name="qkT_p", bufs=1) as qkT_p,
            tc.tile_pool(name="v_p", bufs=1) as v_p,
            tc.tile_pool(name="values_p", bufs=1) as values_p,
            tc.tile_pool(name="pT_p", bufs=6) as pT_p,
            tc.tile_pool(name="lrow_p", bufs=2) as lrow_p,
            tc.tile_pool(name="bctsb_p", bufs=2) as bctsb_p,
            tc.tile_pool(name="vtmp_p", bufs=2) as vtmp_p,
            tc.tile_pool(name="out_p", bufs=6) as out_p,
        ):
            # ---- constants ----
            bias_qk_sb = consts.tile([128, 4], F32)
            vbias_bc = consts.tile([128, 256], F32)
            wo_t = consts.tile([128, 2, D], BF16, name="wo_t")
            tri_sb = consts.tile([128, 128], BF16)
            eye_sb = consts.tile([64, 64], BF16)
            ones_hi = consts.tile([128, 64], BF16)
            nc.vector.memset(ones_hi[64:65, :], 1.0)
            dummy = consts.tile([128, 512], BF16, name="dummy")
            nc.vector.memset(dummy[64:65, :], 1.0)

            # persistent activation tensors
            # one tile per DMA piece: Tile's dependency tracking coarsens
            # multi-writer tiles to whole-tile, which would make the first
            # QKV matmul wait for every piece of w/x instead of its own.
            W_PIECES = ((0, 1), (1, 1), (2, 2), (4, 2), (6, 2))
            X_PIECES = (((0, 1), (1, 1), (2, 2), (4, 2), (6, 2)),) + (
                ((0, 4), (4, 4)),
            ) * (NCH - 1)
            wt_p = {
                k0: xw_p.tile([128, nk, 768], BF16, name=f"wt{k0}")
                for k0, nk in W_PIECES
            }
            xs_p = [
                {
                    k0: xw_p.tile([128, nk, 512], BF16, name=f"x{c}_{k0}")
                    for k0, nk in X_PIECES[c]
                }
                for c in range(NCH)
            ]

            def wt_ref(k):
                for k0, nk in W_PIECES:
                    if k0 <= k < k0 + nk:
                        return wt_p[k0][:, k - k0, :]
                raise AssertionError

            def xs_ref(c, k):
                for k0, nk in X_PIECES[c]:
                    if k0 <= k < k0 + nk:
                        return xs_p[c][k0][:, k - k0, :]
                raise AssertionError
            qkT = [qkT_p.tile([128, S], QK_DT, name=f"qkT{mt}") for mt in range(4)]
            v_sb = [
                v_p.tile([128, H_PER_CORE, 65], BF16, name=f"v{st}")
                for st in range(ST)
            ]
            values = [
                values_p.tile([128, S], BF16, name=f"vals{hp}") for hp in range(2)
            ]

            # ---- phase 0: bulk loads. Each HWDGE trigger costs ~1.3us of
            # the issuing engine's sequencer, so transfers are consolidated
            # into few large 3D-AP DMAs (k-tiles stacked on the free dim),
            # split into 2-tile pieces purely to spread HW queue bandwidth.
            def _src3(handle, row_stride, k0, nk, col0, ncols):
                return bass.AP(
                    tensor=handle,
                    offset=128 * k0 * row_stride + col0,
                    ap=[[row_stride, 128], [128 * row_stride, nk], [1, ncols]],
                )

            def load_w(eng, k0, nk):
                eng.dma_start(wt_p[k0][:], _src3(w, 768, k0, nk, 0, 768))

            def load_x(eng, c, k0, nk):
                eng.dma_start(xs_p[c][k0][:], _src3(xT, S, k0, nk, 512 * c, 512))

            load_w(nc.sync, 0, 1)
            load_x(nc.scalar, 0, 0, 1)
            load_w(nc.gpsimd, 1, 1)
            load_x(nc.sync, 0, 1, 1)
            load_w(nc.scalar, 2, 2)
            load_x(nc.gpsimd, 0, 2, 2)
            load_w(nc.sync, 4, 2)
            load_x(nc.scalar, 0, 4, 2)
            load_w(nc.gpsimd, 6, 2)
            load_x(nc.sync, 0, 6, 2)
            nc.gpsimd.dma_start(vbias_bc[:], _dram_row_bcast(bias_v, 0, 256, 128))
            nc.sync.dma_start(bias_qk_sb[:], bias_qk[:])
            nc.scalar.dma_start(tri_sb[:], tri[:])
            nc.gpsimd.dma_start(eye_sb[:], eye[:])

            # ---- QKV drain helpers ----
            def drain_qk(mt, c, pq, act=False):
                dst = qkT[mt][:, 512 * c : 512 * (c + 1)]
                ctx = (
                    nc.allow_low_precision(reason="bf16 q/k: ~1e-2 on exp weights")
                    if QK_DT == BF16
                    else _nullcontext()
                )
                with ctx:
                    return _drain_qk_inner(mt, dst, pq, act)

            def _drain_qk_inner(mt, dst, pq, act):
                if act:
                    # ScalarE copy-with-bias: frees DVE for the v drains and
                    # starts attention sooner after the prologue.
                    nc.scalar.activation(
                        dst,
                        pq[:],
                        mybir.ActivationFunctionType.Identity,
                        bias=bias_qk_sb[:, mt : mt + 1],
                    )
                else:
                    nc.vector.tensor_scalar(
                        dst,
                        pq[:],
                        bias_qk_sb[:, mt : mt + 1],
                        None,
                        mybir.AluOpType.add,
                    )

            def drain_v(st, pv, eng=None):
                eng = eng or nc.vector
                eng.memset(v_sb[st][:, :, 64:65], 1.0)
                eng.tensor_tensor(
                    v_sb[st][:, :, 0:64],
                    pv[:].rearrange("p (h d) -> p h d", h=H_PER_CORE),
                    vbias_bc[:].rearrange("p (h d) -> p h d", h=H_PER_CORE),
                    mybir.AluOpType.add,
                )

            # ---- output projection for chunk cp (8 psum tiles + stores) ----
            def emit_proj(cp, po_ps, split=False):
                # two 128-row stripes accumulate into one [128, 2, 1024]
                # staging tile -> a single 512KB store trigger per pair
                # (each HWDGE trigger costs ~1.3us of sequencer time).
                # split=True stores each stripe separately: used for the
                # final chunk so the exposed store tail halves.
                for pair in range(2):
                    o_sb = out_p.tile([128, 2, 1024], BF16, name="o_sb")
                    for half in range(2):
                        st = 4 * cp + 2 * pair + half
                        for nh in range(2):
                            po = po_ps.tile([128, 512], F32, name="po")
                            for hp in range(2):
                                nc.tensor.matmul(
                                    po[:],
                                    values[hp][:, 128 * st : 128 * (st + 1)],
                                    wo_t[:, hp, 512 * nh : 512 * (nh + 1)],
                                    start=(hp == 0),
                                    stop=(hp == 1),
                                )
                            dstc = o_sb[:, half, 512 * nh : 512 * (nh + 1)]
                            if nh == 0:
                                nc.vector.tensor_copy(dstc, po[:])
                            else:
                                nc.scalar.copy(dstc, po[:])
                    st0 = 4 * cp + 2 * pair
                    # stores ride SP/Pool queues: a trigger on the ACT queue
                    # would cost ~1.3us of the sequencer that feeds the next
                    # chunk's exp stream.
                    if split:
                        for half in range(2):
                            dma_rr2(
                                out[128 * (st0 + half) : 128 * (st0 + half + 1), :],
                                o_sb[:, half, :],
                            )
                    else:
                        dst = bass.AP(
                            tensor=out,
                            offset=128 * st0 * D,
                            ap=[[D, 128], [128 * D, 2], [1, D]],
                        )
                        dma_rr2(dst, o_sb[:])

            # ---- PE warmup: dummy matmuls run during the initial load
            # wait so the PE is at full p-state when real work starts (the
            # engine runs at half clock for ~3us after any idle period).
            with tc.tile_pool(name="warm", bufs=1, space="PSUM") as warm_ps:
                wps = warm_ps.tile([128, 512], F32, name="wps")
                for _ in range(4):
                    nc.tensor.matmul(
                        wps[0:64, :],
                        dummy[64:65, 0:64],
                        dummy[64:65, :],
                        start=True,
                        stop=True,
                    )

            # ---- prologue: chunk-0 QKV, k-major so it overlaps the loads ----
            with (
                tc.tile_pool(name="pro_q", bufs=1, space="PSUM") as pro_q,
                tc.tile_pool(name="pro_v", bufs=1, space="PSUM") as pro_v,
            ):
                pq0 = [pro_q.tile([128, 512], F32, name=f"pq{mt}") for mt in range(4)]
                pv0 = [pro_v.tile([128, 256], F32, name=f"pv{st}") for st in range(4)]
                for k in range(8):
                    for mt in range(4):
                        nc.tensor.matmul(
                            pq0[mt][:],
                            wt_ref(k)[:, 128 * mt : 128 * (mt + 1)],
                            xs_ref(0, k),
                            start=(k == 0),
                            stop=(k == 7),
                        )
                    for st in range(4):
                        nc.tensor.matmul(
                            pv0[st][:],
                            xs_ref(0, k)[:, 128 * st : 128 * (st + 1)],
                            wt_ref(k)[:, 512:768],
                            start=(k == 0),
                            stop=(k == 7),
                        )
                # remaining loads: issued here so their triggers sit behind
                # the prologue drains in the sequencer streams.
                load_x(nc.sync, 1, 0, 4)
                load_x(nc.scalar, 1, 4, 4)
                nc.gpsimd.dma_start(wo_t[:], _src3(wo, D, 0, 2, 0, D))
                load_x(nc.sync, 2, 0, 4)
                load_x(nc.scalar, 2, 4, 4)
                load_x(nc.gpsimd, 3, 0, 4)
                load_x(nc.sync, 3, 4, 4)
                # Drain order matters doubly: head-0 scores need q01/k01 data
                # AND the first sT psum tile reuses pq0[0]/pq0[1]'s banks.
                # qk drains go on ScalarE (0,2,1,3) while the v drains run
                # concurrently on DVE (Pool can't read PSUM).
                drain_qk(0, 0, pq0[0], act=True)
                drain_qk(1, 0, pq0[1])  # DVE, parallel: frees sT bank 1
                drain_qk(2, 0, pq0[2], act=True)
                drain_qk(3, 0, pq0[3], act=True)
                for st in range(4):
                    drain_v(st, pv0[st])

            with (
                tc.tile_pool(name="bct_ps", bufs=1, space="PSUM") as bct_ps,
                tc.tile_pool(name="mv_ps", bufs=1, space="PSUM") as mv_ps,
            ):
                # deferred normalize: recip runs at head end; the PE bcast +
                # DVE copies are emitted a bit later (under the next head's
                # first score matmuls) so nothing stalls on the reciprocal.
                pending_norm = []

                def emit_norm_tail(h, hp, av, lrow, chunk):
                    bct = bct_ps.tile([128, 512], F32, name="bct")
                    nc.tensor.matmul(
                        bct[0:64, :],
                        ones_hi[64:65, :],
                        lrow[64:65, :],
                        start=True,
                        stop=True,
                    )
                    bct_sb = bctsb_p.tile([128, 512], F32, name="bct_sb")
                    nc.vector.tensor_copy(bct_sb[0:64, :], bct[0:64, :])
                    cols = slice(512 * chunk, 512 * (chunk + 1))
                    if h % 2 == 0:
                        nc.vector.tensor_tensor(
                            values[hp][0:64, cols],
                            av[0:64, :],
                            bct_sb[0:64, :],
                            mybir.AluOpType.mult,
                        )
                    else:
                        vtmp = vtmp_p.tile([128, 512], BF16, name="vtmp")
                        nc.vector.tensor_tensor(
                            vtmp[0:64, :],
                            av[0:64, :],
                            bct_sb[0:64, :],
                            mybir.AluOpType.mult,
                        )
                        mv = mv_ps.tile([128, 512], F32, name="mv")
                        nc.tensor.matmul(
                            mv[64:128, :],
                            eye_sb[:],
                            vtmp[0:64, :],
                            start=True,
                            stop=True,
                        )
                        nc.vector.tensor_copy(values[hp][64:128, cols], mv[64:128, :])

                def flush_norm():
                    while pending_norm:
                        pending_norm.pop(0)()

                # ---- chunk loop ----
                for c in range(NCH):
                    # -- attention for chunk c --
                    with (
                        tc.tile_pool(name=f"sT{c}", bufs=2, space="PSUM") as sT_ps,
                        tc.tile_pool(name=f"av{c}", bufs=2, space="PSUM") as av_ps,
                    ):
                        njt = 4 * c + 4
                        ihi = 512 * (c + 1)
                        npair = njt // 2
                        # flattened (head, pair) pipeline: the next head's
                        # first scores issue before the previous head's last
                        # AV, so the final exp of each head is hidden under
                        # PE work instead of stalling it.
                        avs = {}

                        def emit_av(ent):
                            eh, pcs, ppT = ent
                            for jt, s0, wp in pcs:
                                nc.tensor.matmul(
                                    avs[eh][0:65, 512 - wp : 512],
                                    v_sb[jt][:, eh, :],
                                    ppT[:, s0 : s0 + wp],
                                    start=(jt == 0),
                                    stop=(jt == njt - 1),
                                )

                        def emit_recip(eh):
                            av = avs[eh]
                            lrow = lrow_p.tile([128, 512], BF16, name="lrow")
                            with nc.allow_low_precision(
                                reason="1/l broadcast in bf16; 0.4% on a "
                                "common-mode scale is inside tolerance"
                            ):
                                nc.vector.reciprocal(lrow[64:65, :], av[64:65, :])
                            pending_norm.append(
                                lambda h=eh, hp=eh // 2, av=av, lrow=lrow, c=c: (
                                    emit_norm_tail(h, hp, av, lrow, c)
                                )
                            )

                        pend = []  # AV queue, depth 2: exp gets two pair-
                        # times of slack before its AV issues on PE.

                        def pop_av():
                            ent = pend.pop(0)
                            emit_av(ent[:3])
                            if ent[3]:
                                emit_recip(ent[0])

                        for h in range(H_PER_CORE):
                            hp, hr = h // 2, 64 * (h % 2)
                            q_t = qkT[hp]
                            k_t = qkT[2 + hp]
                            avs[h] = av_ps.tile([128, 512], F32, name=f"av{h}", tag="av")
                            for p in range(npair):
                                jtA, jtB = 2 * p, 2 * p + 1
                                sT = sT_ps.tile([128, 1024], F32, name="sT", tag="sT")
                                pT = pT_p.tile([128, 1024], BF16, name="pT", tag="pT")
                                # widths; diagonal pair 2 packs right-aligned
                                # against col 512 so the exp region stays
                                # contiguous across the PSUM bank boundary.
                                wA = min(512, ihi - 128 * jtA)
                                wB = min(512, ihi - 128 * jtB)
                                sA0 = 512 - wA
                                pieces = ((jtA, sA0, wA), (jtB, 512, wB))
                                for jt, s0, wp in pieces:
                                    nc.tensor.matmul(
                                        sT[:, s0 : s0 + wp],
                                        k_t[hr : hr + 64, 128 * jt : 128 * (jt + 1)],
                                        q_t[hr : hr + 64, ihi - wp : ihi],
                                        start=True,
                                        stop=True,
                                    )
                                nc.scalar.activation(
                                    pT[:, sA0 : 512 + wB],
                                    sT[:, sA0 : 512 + wB],
                                    mybir.ActivationFunctionType.Exp,
                                    scale=float(SCALE),
                                )
                                for jt, s0, wp in pieces:
                                    if jt >= 4 * c:
                                        # diagonal j-tile: mask first 128 cols
                                        nc.vector.tensor_tensor(
                                            pT[:, s0 : s0 + 128],
                                            pT[:, s0 : s0 + 128],
                                            tri_sb[:],
                                            mybir.AluOpType.mult,
                                        )
                                if len(pend) >= 2:
                                    pop_av()
                                if p == min(2, npair - 1):
                                    # one pair after the recip pops: the
                                    # broadcast matmul never waits on it.
                                    flush_norm()
                                pend.append((h, pieces, pT, p == npair - 1))
                        while pend:
                            pop_av()
                        # head 3 must normalize before this chunk's attention
                        # pools close (its av lives in them).
                        flush_norm()

                    # -- QKV chunk c+1, then proj/store chunk c --
                    if c + 1 < NCH:
                        with (
                            tc.tile_pool(name=f"qkv{c}", bufs=2, space="PSUM") as qkv_ps,
                            tc.tile_pool(name=f"vps{c}", bufs=2, space="PSUM") as vps,
                            tc.tile_pool(name=f"pop{c}", bufs=2, space="PSUM") as po_ps,
                        ):
                            for mt in (0, 2, 1, 3):
                                pq = qkv_ps.tile([128, 512], F32, name="pq")
                                for k in range(8):
                                    nc.tensor.matmul(
                                        pq[:],
                                        wt_ref(k)[:, 128 * mt : 128 * (mt + 1)],
                                        xs_ref(c + 1, k),
                                        start=(k == 0),
                                        stop=(k == 7),
                                    )
                                # ScalarE drain: next chunk's first scores
                                # unblock without queueing behind DVE work.
                                drain_qk(mt, c + 1, pq, act=True)
                            for sti in range(4):
                                st = 4 * (c + 1) + sti
                                pv = vps.tile([128, 256], F32, name="pv")
                                for k in range(8):
                                    nc.tensor.matmul(
                                        pv[:],
                                        xs_ref(c + 1, k)[:, 128 * sti : 128 * (sti + 1)],
                                        wt_ref(k)[:, 512:768],
                                        start=(k == 0),
                                        stop=(k == 7),
                                    )
                                drain_v(st, pv)
                            emit_proj(c, po_ps)
                    else:
                        with tc.tile_pool(name="po_f", bufs=4, space="PSUM") as po_ps:
                            emit_proj(c, po_ps, split=True)

    _split_multi_waits(nc)
    return nc


_NC_CACHE = None


def _get_nc():
    global _NC_CACHE
    if _NC_CACHE is None:
        _NC_CACHE = build_bass()
    return _NC_CACHE


def make_in_maps(x, mask, Wqkv, bqkv, Wo, bo):
    x = np.asarray(x, dtype=np.float32)
    Wqkv = np.asarray(Wqkv, dtype=np.float32)
    bqkv = np.asarray(bqkv, dtype=np.float32)
    Wo = np.asarray(Wo, dtype=np.float32)

    import ml_dtypes

    xT = [np.ascontiguousarray(x[b].T).astype(ml_dtypes.bfloat16) for b in range(B)]
    tri = (np.arange(128)[None, :] >= np.arange(128)[:, None]).astype(ml_dtypes.bfloat16)
    eye = np.eye(64, dtype=ml_dtypes.bfloat16)

    in_maps = []
    for c in range(N_CORES):
        b, g = c // 4, c % 4
        heads = [4 * g + h for h in range(H_PER_CORE)]
        # Wqkv columns are per-head interleaved: head H -> q cols
        # 192H..192H+64, k cols 192H+64.., v cols 192H+128..
        iq = np.concatenate([np.arange(192 * H, 192 * H + 64) for H in heads])
        ik = np.concatenate([np.arange(192 * H + 64, 192 * H + 128) for H in heads])
        iv = np.concatenate([np.arange(192 * H + 128, 192 * H + 192) for H in heads])
        w_c = np.ascontiguousarray(
            np.concatenate([Wqkv[:, iq], Wqkv[:, ik], Wqkv[:, iv]], axis=1)
        ).astype(ml_dtypes.bfloat16)
        bias_qk = np.stack(
            [bqkv[iq[:128]], bqkv[iq[128:]], bqkv[ik[:128]], bqkv[ik[128:]]],
            axis=1,
        ).astype(np.float32)
        bias_v = np.ascontiguousarray(bqkv[iv])
        wo_c = np.ascontiguousarray(Wo[256 * g : 256 * (g + 1), :]).astype(
            ml_dtypes.bfloat16
        )
        in_maps.append(
            {
                "xT": xT[b],
                "w": w_c,
                "bias_qk": bias_qk,
                "bias_v": bias_v,
                "wo": wo_c,
                "tri": tri,
                "eye": eye,
            }
        )
    return in_maps


def bench(x, mask, Wqkv, bqkv, Wo, bo, iters=20):
    """Steady-state timing of the NEFF execution via PJRT with
    device-resident inputs. Returns (best_ns, all_ns)."""
    import time

    import jax
    from jax.sharding import Mesh, PartitionSpec
    from jax.experimental.shard_map import shard_map
    from concourse import bass2jax
    from concourse.bass2jax import _bass_exec_p, install_neuronx_cc_hook

    install_neuronx_cc_hook()
    nc = _get_nc()
    in_maps = make_in_maps(x, mask, Wqkv, bqkv, Wo, bo)

    partition_name = nc.partition_id_tensor.name if nc.partition_id_tensor else None
    in_names, out_names, out_avals, zero_shapes = [], [], [], []
    for alloc in nc.m.functions[0].allocations:
        if not isinstance(alloc, mybir.MemoryLocationSet):
            continue
        name = alloc.memorylocations[0].name
        if alloc.kind == "ExternalInput":
            if name != partition_name:
                in_names.append(name)
        elif alloc.kind == "ExternalOutput":
            out_names.append(name)
            shape = tuple(alloc.tensor_shape)
            dtype = mybir.dt.np(alloc.dtype)
            out_avals.append(jax.core.ShapedArray(shape, dtype))
            zero_shapes.append((shape, dtype))
    n_params = len(in_names)
    n_outs = len(out_avals)
    all_in_names = list(in_names) + list(out_names)
    if partition_name is not None:
        all_in_names.append(partition_name)

    def _body(*args):
        operands = list(args)
        if partition_name is not None:
            operands.append(bass2jax.partition_id_tensor())
        outs = _bass_exec_p.bind(
            *operands,
            out_avals=tuple(out_avals),
            in_names=tuple(all_in_names),
            out_names=tuple(out_names),
            lowering_input_output_aliases=(),
            sim_require_finite=True,
            sim_require_nnan=True,
            nc=nc,
        )
        return tuple(outs)

    devices = jax.devices()[:N_CORES]
    mesh = Mesh(np.asarray(devices), ("core",))
    donate = tuple(range(n_params, n_params + n_outs))
    sharded = jax.jit(
        shard_map(
            _body,
            mesh=mesh,
            in_specs=(PartitionSpec("core"),) * (n_params + n_outs),
            out_specs=(PartitionSpec("core"),) * n_outs,
            check_rep=False,
        ),
        donate_argnums=donate,
        keep_unused=True,
    )

    concat_in = [
        np.concatenate(
            [np.asarray(in_maps[c][in_names[i]]) for c in range(N_CORES)], axis=0
        )
        for i in range(n_params)
    ]
    sharding = jax.sharding.NamedSharding(mesh, PartitionSpec("core"))
    dev_in = [jax.device_put(a, sharding) for a in concat_in]

    def make_zeros():
        return [
            jax.device_put(np.zeros((N_CORES * s[0], *s[1:]), dt), sharding)
            for (s, dt) in zero_shapes
        ]

    # Async python-level chaining: each call donates the previous call's
    # outputs as its output buffers; calls pipeline on the device and we
    # only block at the end. Marginal time over the rep count isolates
    # per-execution device time from fixed RPC/dispatch overhead.
    def timed(reps):
        ts = []
        for _ in range(iters):
            outs = make_zeros()
            for z in outs:
                z.block_until_ready()
            t0 = time.perf_counter()
            for _ in range(reps):
                outs = sharded(*dev_in, *outs)
            for o in outs:
                o.block_until_ready()
            ts.append((time.perf_counter() - t0) * 1e9)
        return ts

    r_lo, r_hi = 1, 65
    t_lo = timed(r_lo)
    t_hi = timed(r_hi)
    best = (min(t_hi) - min(t_lo)) / (r_hi - r_lo)
    med = (sorted(t_hi)[len(t_hi) // 2] - sorted(t_lo)[len(t_lo) // 2]) / (
        r_hi - r_lo
    )
    return best, {"lo": t_lo, "hi": t_hi, "marginal_best": best, "marginal_med": med}


def kernel(x, mask, Wqkv, bqkv, Wo, bo, _trace=False):
    nc = _get_nc()
    in_maps = make_in_maps(x, mask, Wqkv, bqkv, Wo, bo)
    res = run_bass_kernel_spmd(nc, in_maps, core_ids=list(range(N_CORES)), trace=_trace)
    partials = [np.asarray(r["out"], dtype=np.float32) for r in res.results]
    bo = np.asarray(bo, dtype=np.float32)
    out = np.empty((B, S, D), dtype=np.float32)
    for b in range(B):
        out[b] = (
            partials[4 * b]
            + partials[4 * b + 1]
            + partials[4 * b + 2]
            + partials[4 * b + 3]
            + bo
        )
    if _trace:
        return out, res
    return out


# revision 7
# speedup vs baseline: 1.8152x; 1.8152x over previous
"""Multi-head causal attention (B=2, S=2048, d_model=1024, H=16) on 8 Trainium2
NeuronCores — chunk-major pipelined version.

Sharding: core c -> batch b = c // 4, head group g = c % 4 (heads 4g..4g+3).
Data-parallel over the batch, tensor-parallel over heads: each core computes
QKV projections for its 4 heads (column-sliced Wqkv), causal attention for
those heads, and a partial output projection (row-sliced Wo). The host sums
the 4 partial outputs per batch and adds the output bias.

Device dataflow (per core): the sequence is processed in 4 query chunks of
512 so input DMA (x column-slices), QKV projection, attention, output
projection, and output stores all overlap chunk-to-chunk:

  prologue: chunk-0 QKV runs k-major, accumulating as x/W k-tiles stream in.
  chunk c:  [attention c] then [QKV c+1 (ch-major) | proj c | stores c].

Attention per (head, chunk): scoresT[j, i] = kT.T @ qT per j-tile pair into
one 2-bank PSUM tile (fp32r), one wide Exp (scale folded) straight to bf16
pT, DVE triangular mask on diagonal blocks, AV (all-bf16, full PE rate at
any width) accumulated over j-tiles into av[65, 512] whose row 64 is the
softmax denominator l via a ones column in v. 1/l comes from a single-
partition DVE reciprocal (no DRAM bounce), broadcast across 64 partitions
with a K=1 matmul; values are normalized straight out of PSUM into bf16.
Odd heads land in partitions 64:128 via an identity-matmul partition move
(no SBUF-to-SBUF DMA). Output projection contracts K=128 per head pair in
bf16; partial outputs stream out per 128-row tile.
"""

import sys

sys.path.insert(0, "/opt/trn_rl_repo")

import numpy as np
from contextlib import nullcontext as _nullcontext

import concourse.bass as bass
import concourse.mybir as mybir
import concourse.tile as tile
from concourse.bass_utils import run_bass_kernel_spmd

F32 = mybir.dt.float32
F32R = mybir.dt.float32r
BF16 = mybir.dt.bfloat16

B, S, D = 2, 2048, 1024
H_TOT = 16
HD = 64
H_PER_CORE = 4
N_CORES = 8
SCALE = 1.0 / np.sqrt(HD)

ST = S // 128   # 16 sequence tiles of 128
NCH = S // 512  # 4 query chunks of 512

# Q/K activations in bf16: logit error ~1e-2 relative on exp weights, well
# inside the 2e-2 budget; buys full-rate narrow matmuls on diagonal tiles,
# halves qkT SBUF and doubles drain throughput.
QK_DT = BF16


def _split_multi_waits(nc):
    """This container's walrus rejects >1 sem wait per instruction. Move
    extra waits onto fresh single-wait NOPs on the same engine, inserted
    immediately before the instruction (same-engine streams are in-order,
    so semantics are unchanged)."""
    n = 0
    for func in nc.m.functions:
        for bb in func.blocks:
            i = 0
            while i < len(bb.instructions):
                ins = bb.instructions[i]
                si = ins.sync_info
                if si is not None and si.on_wait and len(si.on_wait) > 1:
                    waits = list(si.on_wait)
                    si.on_wait = [waits[-1]]
                    eng = nc.engines[ins.engine]
                    nops = []
                    for w in waits[:-1]:
                        ni = eng.nop(nofuse=True, hint="wait_split").ins
                        if ni.sync_info is None:
                            ni.sync_info = mybir.SyncInfo(on_wait=[w], on_update=[])
                        else:
                            ni.sync_info.on_wait = [w]
                        nops.append(ni)
                    for ni in nops:
                        for f2 in nc.m.functions:
                            for bb2 in f2.blocks:
                                if ni in bb2.instructions:
                                    bb2.instructions.remove(ni)
                    for k, ni in enumerate(nops):
                        bb.instructions.insert(i + k, ni)
                    i += len(nops)
                    n += len(nops)
                i += 1
    return n


def _dram_row_bcast(handle, offset_elems, width, parts):
    """AP that broadcasts a DRAM row of `width` elems across `parts` partitions."""
    return bass.AP(tensor=handle, offset=offset_elems, ap=[[0, parts], [1, width]])


def build_bass():
    nc = bass.Bass()

    xT = nc.dram_tensor("xT", [D, S], BF16, kind="ExternalInput")
    w = nc.dram_tensor("w", [D, 768], BF16, kind="ExternalInput")
    bias_qk = nc.dram_tensor("bias_qk", [128, 4], F32, kind="ExternalInput")
    bias_v = nc.dram_tensor("bias_v", [256], F32, kind="ExternalInput")
    wo = nc.dram_tensor("wo", [256, D], BF16, kind="ExternalInput")
    tri = nc.dram_tensor("tri", [128, 128], BF16, kind="ExternalInput")
    eye = nc.dram_tensor("eye", [64, 64], BF16, kind="ExternalInput")
    out = nc.dram_tensor("out", [S, D], BF16, kind="ExternalOutput")

    # Per-queue DMA bandwidth is low in this environment; round-robin bulk
    # transfers across all three DMA-capable queues (SP-HWDGE, ACT-HWDGE,
    # Pool-SWDGE). ACT's queue only carries loads issued while ScalarE is
    # idle plus stores whose staging copy ran on ScalarE (in-order, no
    # waits), so it never stalls the exp stream.
    _dma_engines = [nc.sync, nc.scalar, nc.gpsimd]
    _dma_i = [0]

    def dma_rr(out_ap, in_ap):
        e = _dma_engines[_dma_i[0] % len(_dma_engines)]
        _dma_i[0] += 1
        return e.dma_start(out_ap, in_ap)

    _dma2_engines = [nc.sync, nc.gpsimd]
    _dma2_i = [0]

    def dma_rr2(out_ap, in_ap):
        e = _dma2_engines[_dma2_i[0] % len(_dma2_engines)]
        _dma2_i[0] += 1
        return e.dma_start(out_ap, in_ap)

    with tile.TileContext(nc) as tc:
        with (
            tc.tile_pool(name="consts", bufs=1) as consts,
            tc.tile_pool(name="xw_p", bufs=1) as xw_p,
            tc.tile_pool(name="qkT_p", bufs=1) as qkT_p,
            tc.tile_pool(name="v_p", bufs=1) as v_p,
            tc.tile_pool(name="values_p", bufs=1) as values_p,
            tc.tile_pool(name="pT_p", bufs=8) as pT_p,
            tc.tile_pool(name="lrow_p", bufs=2) as lrow_p,
            tc.tile_pool(name="bctsb_p", bufs=4) as bctsb_p,
            tc.tile_pool(name="vtmp_p", bufs=4) as vtmp_p,
            tc.tile_pool(name="out_p", bufs=8) as out_p,
        ):
            # ---- constants ----
            bias_qk_sb = consts.tile([128, 4], F32)
            vbias_bc = consts.tile([128, 256], F32)
            wo_t = consts.tile([128, 2, D], BF16, name="wo_t")
            tri_sb = consts.tile([128, 128], BF16)
            eye_sb = consts.tile([64, 64], BF16)
            ones_hi = consts.tile([128, 64], BF16)
            nc.vector.memset(ones_hi[64:65, :], 1.0)
            dummy = consts.tile([128, 512], BF16, name="dummy")
            nc.vector.memset(dummy[64:65, :], 1.0)

            # persistent activation tensors
            # one tile per DMA piece: Tile's dependency tracking coarsens
            # multi-writer tiles to whole-tile, which would make the first
            # QKV matmul wait for every piece of w/x instead of its own.
            W_PIECES = ((0, 1), (1, 1), (2, 2), (4, 2), (6, 2))
            X_PIECES = (((0, 1), (1, 1), (2, 2), (4, 2), (6, 2)),) + (
                ((0, 4), (4, 4)),
            ) * (NCH - 1)
            wt_p = {
                k0: xw_p.tile([128, nk, 768], BF16, name=f"wt{k0}")
                for k0, nk in W_PIECES
            }
            xs_p = [
                {
                    k0: xw_p.tile([128, nk, 512], BF16, name=f"x{c}_{k0}")
                    for k0, nk in X_PIECES[c]
                }
                for c in range(NCH)
            ]

            def wt_ref(k):
                for k0, nk in W_PIECES:
                    if k0 <= k < k0 + nk:
                        return wt_p[k0][:, k - k0, :]
                raise AssertionError

            def xs_ref(c, k):
                for k0, nk in X_PIECES[c]:
                    if k0 <= k < k0 + nk:
                        return xs_p[c][k0][:, k - k0, :]
                raise AssertionError
            qkT = [qkT_p.tile([128, S], QK_DT, name=f"qkT{mt}") for mt in range(4)]
            v_sb = [
                v_p.tile([128, H_PER_CORE, 65], BF16, name=f"v{st}")
                for st in range(ST)
            ]
            values = [
                values_p.tile([128, S], BF16, name=f"vals{hp}") for hp in range(2)
            ]

            # ---- phase 0: bulk loads. Each HWDGE trigger costs ~1.3us of
            # the issuing engine's sequencer, so transfers are consolidated
            # into few large 3D-AP DMAs (k-tiles stacked on the free dim),
            # split into 2-tile pieces purely to spread HW queue bandwidth.
            def _src3(handle, row_stride, k0, nk, col0, ncols):
                return bass.AP(
                    tensor=handle,
                    offset=128 * k0 * row_stride + col0,
                    ap=[[row_stride, 128], [128 * row_stride, nk], [1, ncols]],
                )

            def load_w(eng, k0, nk):
                eng.dma_start(wt_p[k0][:], _src3(w, 768, k0, nk, 0, 768))

            def load_x(eng, c, k0, nk):
                eng.dma_start(xs_p[c][k0][:], _src3(xT, S, k0, nk, 512 * c, 512))

            load_w(nc.sync, 0, 1)
            load_x(nc.scalar, 0, 0, 1)
            load_w(nc.gpsimd, 1, 1)
            load_x(nc.sync, 0, 1, 1)
            load_w(nc.scalar, 2, 2)
            load_x(nc.gpsimd, 0, 2, 2)
            load_w(nc.sync, 4, 2)
            load_x(nc.scalar, 0, 4, 2)
            load_w(nc.gpsimd, 6, 2)
            load_x(nc.sync, 0, 6, 2)
            nc.gpsimd.dma_start(vbias_bc[:], _dram_row_bcast(bias_v, 0, 256, 128))
            nc.sync.dma_start(bias_qk_sb[:], bias_qk[:])
            nc.scalar.dma_start(tri_sb[:], tri[:])
            nc.gpsimd.dma_start(eye_sb[:], eye[:])

            # ---- QKV drain helpers ----
            def drain_qk(mt, c, pq, act=False):
                dst = qkT[mt][:, 512 * c : 512 * (c + 1)]
                ctx = (
                    nc.allow_low_precision(reason="bf16 q/k: ~1e-2 on exp weights")
                    if QK_DT == BF16
                    else _nullcontext()
                )
                with ctx:
                    return _drain_qk_inner(mt, dst, pq, act)

            def _drain_qk_inner(mt, dst, pq, act):
                if act:
                    # ScalarE copy-with-bias: frees DVE for the v drains and
                    # starts attention sooner after the prologue.
                    nc.scalar.activation(
                        dst,
                        pq[:],
                        mybir.ActivationFunctionType.Identity,
                        bias=bias_qk_sb[:, mt : mt + 1],
                    )
                else:
                    nc.vector.tensor_scalar(
                        dst,
                        pq[:],
                        bias_qk_sb[:, mt : mt + 1],
                        None,
                        mybir.AluOpType.add,
                    )

            def drain_v(st, pv, eng=None):
                eng = eng or nc.vector
                eng.memset(v_sb[st][:, :, 64:65], 1.0)
                eng.tensor_tensor(
                    v_sb[st][:, :, 0:64],
                    pv[:].rearrange("p (h d) -> p h d", h=H_PER_CORE),
                    vbias_bc[:].rearrange("p (h d) -> p h d", h=H_PER_CORE),
                    mybir.AluOpType.add,
                )

            # ---- output projection for chunk cp (8 psum tiles + stores) ----
            def emit_proj(cp, po_ps, split=False):
                # two 128-row stripes accumulate into one [128, 2, 1024]
                # staging tile -> a single 512KB store trigger per pair
                # (each HWDGE trigger costs ~1.3us of sequencer time).
                # split=True stores each stripe separately: used for the
                # final chunk so the exposed store tail halves.
                for pair in range(2):
                    o_sb = out_p.tile([128, 2, 1024], BF16, name="o_sb")
                    for half in range(2):
                        st = 4 * cp + 2 * pair + half
                        for nh in range(2):
                            po = po_ps.tile([128, 512], F32, name="po")
                            for hp in range(2):
                                nc.tensor.matmul(
                                    po[:],
                                    values[hp][:, 128 * st : 128 * (st + 1)],
                                    wo_t[:, hp, 512 * nh : 512 * (nh + 1)],
                                    start=(hp == 0),
                                    stop=(hp == 1),
                                )
                            dstc = o_sb[:, half, 512 * nh : 512 * (nh + 1)]
                            if nh == 0:
                                nc.vector.tensor_copy(dstc, po[:])
                            else:
                                nc.scalar.copy(dstc, po[:])
                    st0 = 4 * cp + 2 * pair
                    # stores ride SP/Pool queues: a trigger on the ACT queue
                    # would cost ~1.3us of the sequencer that feeds the next
                    # chunk's exp stream.
                    if split:
                        for half in range(2):
                            dma_rr2(
                                out[128 * (st0 + half) : 128 * (st0 + half + 1), :],
                                o_sb[:, half, :],
                            )
                    else:
                        dst = bass.AP(
                            tensor=out,
                            offset=128 * st0 * D,
                            ap=[[D, 128], [128 * D, 2], [1, D]],
                        )
                        dma_rr2(dst, o_sb[:])

            # ---- PE warmup: dummy matmuls run during the initial load
            # wait so the PE is at full p-state when real work starts (the
            # engine runs at half clock for ~3us after any idle period).
            with tc.tile_pool(name="warm", bufs=1, space="PSUM") as warm_ps:
                wps = warm_ps.tile([128, 512], F32, name="wps")
                for _ in range(4):
                    nc.tensor.matmul(
                        wps[0:64, :],
                        dummy[64:65, 0:64],
                        dummy[64:65, :],
                        start=True,
                        stop=True,
                    )

            # ---- prologue: chunk-0 QKV, k-major so it overlaps the loads ----
            with (
                tc.tile_pool(name="pro_q", bufs=1, space="PSUM") as pro_q,
                tc.tile_pool(name="pro_v", bufs=1, space="PSUM") as pro_v,
            ):
                pq0 = [pro_q.tile([128, 512], F32, name=f"pq{mt}") for mt in range(4)]
                pv0 = [pro_v.tile([128, 256], F32, name=f"pv{st}") for st in range(4)]
                for k in range(8):
                    for mt in range(4):
                        nc.tensor.matmul(
                            pq0[mt][:],
                            wt_ref(k)[:, 128 * mt : 128 * (mt + 1)],
                            xs_ref(0, k),
                            start=(k == 0),
                            stop=(k == 7),
                        )
                    for st in range(4):
                        nc.tensor.matmul(
                            pv0[st][:],
                            xs_ref(0, k)[:, 128 * st : 128 * (st + 1)],
                            wt_ref(k)[:, 512:768],
                            start=(k == 0),
                            stop=(k == 7),
                        )
                # remaining loads: issued here so their triggers sit behind
                # the prologue drains in the sequencer streams.
                load_x(nc.sync, 1, 0, 4)
                load_x(nc.scalar, 1, 4, 4)
                nc.gpsimd.dma_start(wo_t[:], _src3(wo, D, 0, 2, 0, D))
                load_x(nc.sync, 2, 0, 4)
                load_x(nc.scalar, 2, 4, 4)
                load_x(nc.gpsimd, 3, 0, 4)
                load_x(nc.sync, 3, 4, 4)
                # Drain order matters doubly: head-0 scores need q01/k01 data
                # AND the first sT psum tile reuses pq0[0]/pq0[1]'s banks.
                # qk drains go on ScalarE (0,2,1,3) while the v drains run
                # concurrently on DVE (Pool can't read PSUM).
                drain_qk(0, 0, pq0[0], act=True)
                drain_qk(1, 0, pq0[1])  # DVE, parallel: frees sT bank 1
                drain_qk(2, 0, pq0[2], act=True)
                drain_qk(3, 0, pq0[3], act=True)
                for st in range(4):
                    drain_v(st, pv0[st])

            with (
                tc.tile_pool(name="bct_ps", bufs=1, space="PSUM") as bct_ps,
                tc.tile_pool(name="mv_ps", bufs=1, space="PSUM") as mv_ps,
            ):
                # deferred normalize: recip runs at head end; the PE bcast +
                # DVE copies are emitted a bit later (under the next head's
                # first score matmuls) so nothing stalls on the reciprocal.
                pending_norm = []

                def emit_norm_tail(h, hp, av, lrow, chunk):
                    bct = bct_ps.tile([128, 512], F32, name="bct")
                    nc.tensor.matmul(
                        bct[0:64, :],
                        ones_hi[64:65, :],
                        lrow[64:65, :],
                        start=True,
                        stop=True,
                    )
                    bct_sb = bctsb_p.tile([128, 512], F32, name="bct_sb")
                    nc.vector.tensor_copy(bct_sb[0:64, :], bct[0:64, :])
                    cols = slice(512 * chunk, 512 * (chunk + 1))
                    if h % 2 == 0:
                        nc.vector.tensor_tensor(
                            values[hp][0:64, cols],
                            av[0:64, :],
                            bct_sb[0:64, :],
                            mybir.AluOpType.mult,
                        )
                    else:
                        vtmp = vtmp_p.tile([128, 512], BF16, name="vtmp")
                        nc.vector.tensor_tensor(
                            vtmp[0:64, :],
                            av[0:64, :],
                            bct_sb[0:64, :],
                            mybir.AluOpType.mult,
                        )
                        mv = mv_ps.tile([128, 512], F32, name="mv")
                        nc.tensor.matmul(
                            mv[64:128, :],
                            eye_sb[:],
                            vtmp[0:64, :],
                            start=True,
                            stop=True,
                        )
                        nc.vector.tensor_copy(values[hp][64:128, cols], mv[64:128, :])

                def flush_norm():
                    while pending_norm:
                        pending_norm.pop(0)()

                # ---- chunk loop ----
                for c in range(NCH):
                    # -- attention for chunk c --
                    with (
                        tc.tile_pool(name=f"sT{c}", bufs=2, space="PSUM") as sT_ps,
                        tc.tile_pool(name=f"av{c}", bufs=2, space="PSUM") as av_ps,
                    ):
                        njt = 4 * c + 4
                        ihi = 512 * (c + 1)
                        npair = njt // 2
                        # flattened (head, pair) pipeline: the next head's
                        # first scores issue before the previous head's last
                        # AV, so the final exp of each head is hidden under
                        # PE work instead of stalling it.
                        avs = {}

                        def emit_av(ent):
                            eh, pcs, ppT = ent
                            for jt, s0, wp in pcs:
                                nc.tensor.matmul(
                                    avs[eh][0:65, 512 - wp : 512],
                                    v_sb[jt][:, eh, :],
                                    ppT[:, s0 : s0 + wp],
                                    start=(jt == 0),
                                    stop=(jt == njt - 1),
                                )

                        def emit_recip(eh):
                            av = avs[eh]
                            lrow = lrow_p.tile([128, 512], BF16, name="lrow")
                            with nc.allow_low_precision(
                                reason="1/l broadcast in bf16; 0.4% on a "
                                "common-mode scale is inside tolerance"
                            ):
                                nc.vector.reciprocal(lrow[64:65, :], av[64:65, :])
                            pending_norm.append(
                                lambda h=eh, hp=eh // 2, av=av, lrow=lrow, c=c: (
                                    emit_norm_tail(h, hp, av, lrow, c)
                                )
                            )

                        pend = []  # AV queue, depth 2: exp gets two pair-
                        # times of slack before its AV issues on PE.

                        def pop_av():
                            ent = pend.pop(0)
                            emit_av(ent[:3])
                            if ent[3]:
                                emit_recip(ent[0])

                        for h in range(H_PER_CORE):
                            hp, hr = h // 2, 64 * (h % 2)
                            q_t = qkT[hp]
                            k_t = qkT[2 + hp]
                            avs[h] = av_ps.tile([128, 512], F32, name=f"av{h}", tag="av")
                            for p in range(npair):
                                jtA, jtB = 2 * p, 2 * p + 1
                                sT = sT_ps.tile([128, 1024], F32, name="sT", tag="sT")
                                pT = pT_p.tile([128, 1024], BF16, name="pT", tag="pT")
                                # widths; diagonal pair 2 packs right-aligned
                                # against col 512 so the exp region stays
                                # contiguous across the PSUM bank boundary.
                                wA = min(512, ihi - 128 * jtA)
                                wB = min(512, ihi - 128 * jtB)
                                sA0 = 512 - wA
                                pieces = ((jtA, sA0, wA), (jtB, 512, wB))
                                for jt, s0, wp in pieces:
                                    nc.tensor.matmul(
                                        sT[:, s0 : s0 + wp],
                                        k_t[hr : hr + 64, 128 * jt : 128 * (jt + 1)],
                                        q_t[hr : hr + 64, ihi - wp : ihi],
                                        start=True,
                                        stop=True,
                                    )
                                nc.scalar.activation(
                                    pT[:, sA0 : 512 + wB],
                                    sT[:, sA0 : 512 + wB],
                                    mybir.ActivationFunctionType.Exp,
                                    scale=float(SCALE),
                                )
                                for jt, s0, wp in pieces:
                                    if jt >= 4 * c:
                                        # diagonal j-tile: mask first 128 cols
                                        nc.vector.tensor_tensor(
                                            pT[:, s0 : s0 + 128],
                                            pT[:, s0 : s0 + 128],
                                            tri_sb[:],
                                            mybir.AluOpType.mult,
                                        )
                                if len(pend) >= 2:
                                    pop_av()
                                if p == min(2, npair - 1):
                                    # one pair after the recip pops: the
                                    # broadcast matmul never waits on it.
                                    flush_norm()
                                pend.append((h, pieces, pT, p == npair - 1))
                        while pend:
                            pop_av()
                        # head 3 must normalize before this chunk's attention
                        # pools close (its av lives in them).
                        flush_norm()

                    # -- QKV chunk c+1, then proj/store chunk c --
                    if c + 1 < NCH:
                        with (
                            tc.tile_pool(name=f"qkv{c}", bufs=2, space="PSUM") as qkv_ps,
                            tc.tile_pool(name=f"vps{c}", bufs=2, space="PSUM") as vps,
                            tc.tile_pool(name=f"pop{c}", bufs=2, space="PSUM") as po_ps,
                        ):
                            for mt in (0, 2, 1, 3):
                                pq = qkv_ps.tile([128, 512], F32, name="pq")
                                for k in range(8):
                                    nc.tensor.matmul(
                                        pq[:],
                                        wt_ref(k)[:, 128 * mt : 128 * (mt + 1)],
                                        xs_ref(c + 1, k),
                                        start=(k == 0),
                                        stop=(k == 7),
                                    )
                                # ScalarE drain: next chunk's first scores
                                # unblock without queueing behind DVE work.
                                drain_qk(mt, c + 1, pq, act=True)
                            for sti in range(4):
                                st = 4 * (c + 1) + sti
                                pv = vps.tile([128, 256], F32, name="pv")
                                for k in range(8):
                                    nc.tensor.matmul(
                                        pv[:],
                                        xs_ref(c + 1, k)[:, 128 * sti : 128 * (sti + 1)],
                                        wt_ref(k)[:, 512:768],
                                        start=(k == 0),
                                        stop=(k == 7),
                                    )
                                drain_v(st, pv)
                            emit_proj(c, po_ps)
                    else:
                        with tc.tile_pool(name="po_f", bufs=4, space="PSUM") as po_ps:
                            emit_proj(c, po_ps, split=True)

    _split_multi_waits(nc)
    return nc


_NC_CACHE = None


def _get_nc():
    global _NC_CACHE
    if _NC_CACHE is None:
        _NC_CACHE = build_bass()
    return _NC_CACHE


def make_in_maps(x, mask, Wqkv, bqkv, Wo, bo):
    x = np.asarray(x, dtype=np.float32)
    Wqkv = np.asarray(Wqkv, dtype=np.float32)
    bqkv = np.asarray(bqkv, dtype=np.float32)
    Wo = np.asarray(Wo, dtype=np.float32)

    import ml_dtypes

    xT = [np.ascontiguousarray(x[b].T).astype(ml_dtypes.bfloat16) for b in range(B)]
    tri = (np.arange(128)[None, :] >= np.arange(128)[:, None]).astype(ml_dtypes.bfloat16)
    eye = np.eye(64, dtype=ml_dtypes.bfloat16)

    in_maps = []
    for c in range(N_CORES):
        b, g = c // 4, c % 4
        heads = [4 * g + h for h in range(H_PER_CORE)]
        # Wqkv columns are per-head interleaved: head H -> q cols
        # 192H..192H+64, k cols 192H+64.., v cols 192H+128..
        iq = np.concatenate([np.arange(192 * H, 192 * H + 64) for H in heads])
        ik = np.concatenate([np.arange(192 * H + 64, 192 * H + 128) for H in heads])
        iv = np.concatenate([np.arange(192 * H + 128, 192 * H + 192) for H in heads])
        w_c = np.ascontiguousarray(
            np.concatenate([Wqkv[:, iq], Wqkv[:, ik], Wqkv[:, iv]], axis=1)
        ).astype(ml_dtypes.bfloat16)
        bias_qk = np.stack(
            [bqkv[iq[:128]], bqkv[iq[128:]], bqkv[ik[:128]], bqkv[ik[128:]]],
            axis=1,
        ).astype(np.float32)
        bias_v = np.ascontiguousarray(bqkv[iv])
        wo_c = np.ascontiguousarray(Wo[256 * g : 256 * (g + 1), :]).astype(
            ml_dtypes.bfloat16
        )
        in_maps.append(
            {
                "xT": xT[b],
                "w": w_c,
                "bias_qk": bias_qk,
                "bias_v": bias_v,
                "wo": wo_c,
                "tri": tri,
                "eye": eye,
            }
        )
    return in_maps


def bench(x, mask, Wqkv, bqkv, Wo, bo, iters=20):
    """Steady-state timing of the NEFF execution via PJRT with
    device-resident inputs. Returns (best_ns, all_ns)."""
    import time

    import jax
    from jax.sharding import Mesh, PartitionSpec
    from jax.experimental.shard_map import shard_map
    from concourse import bass2jax
    from concourse.bass2jax import _bass_exec_p, install_neuronx_cc_hook

    install_neuronx_cc_hook()
    nc = _get_nc()
    in_maps = make_in_maps(x, mask, Wqkv, bqkv, Wo, bo)

    partition_name = nc.partition_id_tensor.name if nc.partition_id_tensor else None
    in_names, out_names, out_avals, zero_shapes = [], [], [], []
    for alloc in nc.m.functions[0].allocations:
        if not isinstance(alloc, mybir.MemoryLocationSet):
            continue
        name = alloc.memorylocations[0].name
        if alloc.kind == "ExternalInput":
            if name != partition_name:
                in_names.append(name)
        elif alloc.kind == "ExternalOutput":
            out_names.append(name)
            shape = tuple(alloc.tensor_shape)
            dtype = mybir.dt.np(alloc.dtype)
            out_avals.append(jax.core.ShapedArray(shape, dtype))
            zero_shapes.append((shape, dtype))
    n_params = len(in_names)
    n_outs = len(out_avals)
    all_in_names = list(in_names) + list(out_names)
    if partition_name is not None:
        all_in_names.append(partition_name)

    def _body(*args):
        operands = list(args)
        if partition_name is not None:
            operands.append(bass2jax.partition_id_tensor())
        outs = _bass_exec_p.bind(
            *operands,
            out_avals=tuple(out_avals),
            in_names=tuple(all_in_names),
            out_names=tuple(out_names),
            lowering_input_output_aliases=(),
            sim_require_finite=True,
            sim_require_nnan=True,
            nc=nc,
        )
        return tuple(outs)

    devices = jax.devices()[:N_CORES]
    mesh = Mesh(np.asarray(devices), ("core",))
    donate = tuple(range(n_params, n_params + n_outs))
    sharded = jax.jit(
        shard_map(
            _body,
            mesh=mesh,
            in_specs=(PartitionSpec("core"),) * (n_params + n_outs),
            out_specs=(PartitionSpec("core"),) * n_outs,
            check_rep=False,
        ),
        donate_argnums=donate,
        keep_unused=True,
    )

    concat_in = [
        np.concatenate(
            [np.asarray(in_maps[c][in_names[i]]) for c in range(N_CORES)], axis=0
        )
        for i in range(n_params)
    ]
    sharding = jax.sharding.NamedSharding(mesh, PartitionSpec("core"))
    dev_in = [jax.device_put(a, sharding) for a in concat_in]

    def make_zeros():
        return [
            jax.device_put(np.zeros((N_CORES * s[0], *s[1:]), dt), sharding)
            for (s, dt) in zero_shapes
        ]

    # Async python-level chaining: each call donates the previous call's
    # outputs as its output buffers; calls pipeline on the device and we
    # only block at the end. Marginal time over the rep count isolates
    # per-execution device time from fixed RPC/dispatch overhead.
    def timed(reps):
        ts = []
        for _ in range(iters):
            outs = make_zeros()
            for z in outs:
                z.block_until_ready()
            t0 = time.perf_counter()
            for _ in range(reps):
                outs = sharded(*dev_in, *outs)
            for o in outs:
                o.block_until_ready()
            ts.append((time.perf_counter() - t0) * 1e9)
        return ts

    r_lo, r_hi = 1, 65
    t_lo = timed(r_lo)
    t_hi = timed(r_hi)
    best = (min(t_hi) - min(t_lo)) / (r_hi - r_lo)
    med = (sorted(t_hi)[len(t_hi) // 2] - sorted(t_lo)[len(t_lo) // 2]) / (
        r_hi - r_lo
    )
    return best, {"lo": t_lo, "hi": t_hi, "marginal_best": best, "marginal_med": med}


def kernel(x, mask, Wqkv, bqkv, Wo, bo, _trace=False):
    nc = _get_nc()
    in_maps = make_in_maps(x, mask, Wqkv, bqkv, Wo, bo)
    res = run_bass_kernel_spmd(nc, in_maps, core_ids=list(range(N_CORES)), trace=_trace)
    partials = [np.asarray(r["out"], dtype=np.float32) for r in res.results]
    bo = np.asarray(bo, dtype=np.float32)
    out = np.empty((B, S, D), dtype=np.float32)
    for b in range(B):
        out[b] = (
            partials[4 * b]
            + partials[4 * b + 1]
            + partials[4 * b + 2]
            + partials[4 * b + 3]
            + bo
        )
    if _trace:
        return out, res
    return out
